# revision 33
# baseline (speedup 1.0000x reference)
"""Trainium2 Bass kernel for a 2-layer GAT (heads=1) + linear classifier.

Strategy (8 NeuronCores, SPMD single program):
  - Destination-node sharding: core c owns dst nodes [c*SHARD_PAD,
    (c+1)*SHARD_PAD).  Both layers' node transforms are SHARDED: each core
    computes its own h rows and the full gather tables (hg1/hg2) are
    assembled with chunked AllGather collectives (chunk-major layout);
    gather indices are host-permuted into chunk-major table positions.
  - Gather tables, 192-f32 (768B) rows: [alpha_src, h(128), 1, pad...].
    alpha_src/alpha_dst come from the node matmul via augmented weights
    [W@a_src | W | W@a_dst] (host-computed).  The constant ones column
    (softmax denominator accumulator) is pre-written into the DRAM shard
    tables once.  Per-core alpha_dst tables ([SHARD_PAD, 64] f32, value at
    col 0) serve the per-edge alpha_dst[dst] gather with local indices.
  - Edge phase: edges sorted by dst, grouped in 128-edge tiles per dst
    block; dma_gather needs int16 indices, so the node table is split in
    32768-row ranges and each block's edges are grouped by source range.
    Per supergroup (4 blocks): one dma_gather per range for h-rows, one
    for alpha_dst.  S_ex[p,t,j] = (iota_j == dstcol[p,t]) * exp(lrelu(
    as + ad)) is built for ALL tiles with two broadcast tensor_tensor ops,
    then one TensorE matmul per tile accumulates [dst, h|den] into PSUM
    (max-subtraction skipped - mathematically identity, exponents small).
  - The measured wall clock is dominated by host->device transfer and by
    per-instruction dispatch, so the design minimizes BOTH shipped bytes
    (index tables ship unreplicated [16, n/16] int16 + int8 dst columns,
    expanded on device with broadcast-AP DMAs; x ships bf16 sharded) and
    instruction count (f32 end-to-end avoids dtype-conversion costs,
    supergroup-batched DVE, single-buffer pools, merged row-build copies).
  - Output: per-block PE transpose + matmul with Wlin, bf16 row DMA.
"""

import numpy as np

RANGE = 32768  # dma_gather int16 index limit per sub-table


# ---------------------------------------------------------------- config ----
class Cfg:
    def __init__(self, N=100000, F=128, HID=128, C=10, NC=8, neg_slope=0.2,
                 sg_blocks=4, ag_chunks=7):
        assert F == 128 and HID == 128
        self.N, self.F, self.HID, self.C, self.NC = N, F, HID, C, NC
        self.neg = neg_slope
        self.W = 192                        # row elems (f32): 768B
        self.ADW = 64                       # alpha_dst table row elems (f32)
        shard = (N + NC - 1) // NC
        self.SHARD_PAD = ((shard + 127) // 128) * 128
        self.NPAD = self.SHARD_PAD * NC
        self.NBLK = self.SHARD_PAD // 128   # dst blocks per core
        self.SG_BLOCKS = sg_blocks
        self.NR = (self.NPAD + RANGE - 1) // RANGE
        ag = min(ag_chunks, self.NBLK)
        while self.NBLK % ag:
            ag -= 1
        self.AG_CHUNKS = ag
        self.CBR = (self.NBLK // ag) * 128  # rows per AG chunk per core


def _wrap16(idx_list):
    """int16 idx list (len % 16 == 0) -> [16, len//16] compact wrapped array.

    dma_gather wants this replicated to [128, len//16] (8 GPSIMD cores x 16
    partitions); the replication is rebuilt on device to save host->device
    transfer."""
    a = np.asarray(idx_list, dtype=np.int16)
    return np.ascontiguousarray(a.reshape(-1, 16).T)


# ---------------------------------------------------------- preprocessing ----
def _layer_meta(cfg, src_pos, dst):
    """Build per-layer tile structure + per-core arrays.

    src_pos: [NC] per-core arrays of table positions of edge sources
    dst:     [NC] per-core global dst, sorted (defines blocks)
    """
    NC = cfg.NC

    # per core, per block, per range: edge position lists
    per = []
    for c in range(NC):
        loc = dst[c] - np.int64(c) * cfg.SHARD_PAD
        blk = loc // 128
        rng = src_pos[c] // RANGE
        order = np.lexsort((rng, blk))
        key = blk[order] * cfg.NR + rng[order]
        bounds = np.searchsorted(key, np.arange(cfg.NBLK * cfg.NR + 1))
        lists = [[order[bounds[b * cfg.NR + r]:bounds[b * cfg.NR + r + 1]]
                  for r in range(cfg.NR)] for b in range(cfg.NBLK)]
        per.append(lists)

    # uniform tiles per (block, range)
    Tbr = np.zeros((cfg.NBLK, cfg.NR), dtype=np.int64)
    for b in range(cfg.NBLK):
        for r in range(cfg.NR):
            mx = max(len(per[c][b][r]) for c in range(NC))
            Tbr[b][r] = (mx + 127) // 128
        if Tbr[b].sum() == 0:
            Tbr[b][0] = 1  # empty block still needs one (pad) tile

    # supergroups of SG_BLOCKS consecutive blocks
    sgs = []
    slot0 = 0
    gcol0 = 0
    for s0 in range(0, cfg.NBLK, cfg.SG_BLOCKS):
        blocks = list(range(s0, min(s0 + cfg.SG_BLOCKS, cfg.NBLK)))
        calls = []      # per range: (gidx col_off, ncols, T_r, chunk_off)
        chunk_off = 0
        bl_tiles = {b: [] for b in blocks}  # block -> slot indices (in sg)
        for r in range(cfg.NR):
            T_r = int(sum(Tbr[b][r] for b in blocks))
            calls.append((gcol0, T_r * 8, T_r, chunk_off))
            o = chunk_off
            for b in blocks:
                for _i in range(int(Tbr[b][r])):
                    bl_tiles[b].append(o)
                    o += 1
            gcol0 += T_r * 8
            chunk_off += T_r
        sgs.append(dict(slot0=slot0, nt=chunk_off, calls=calls,
                        blocks=[(b, bl_tiles[b]) for b in blocks]))
        slot0 += chunk_off
    T_total = slot0
    CG = gcol0

    cores = []
    for c in range(NC):
        gidx = np.zeros((16, CG), dtype=np.int16)
        didx = np.zeros((16, T_total * 8), dtype=np.int16)
        dcol = np.full((128, T_total), -1, dtype=np.int8)
        for sg in sgs:
            for r in range(cfg.NR):
                col0, ncols, T_r, chunk_off = sg["calls"][r]
                if T_r == 0:
                    continue
                idx = np.zeros(T_r * 128, dtype=np.int64)
                dloc = np.zeros(T_r * 128, dtype=np.int64)
                dcl = np.full(T_r * 128, -1, dtype=np.int64)
                o = 0
                for b, _t in sg["blocks"]:
                    e = per[c][b][r]
                    n = len(e)
                    idx[o:o + n] = src_pos[c][e] - r * RANGE
                    dl = dst[c][e] - np.int64(c) * cfg.SHARD_PAD
                    dloc[o:o + n] = dl
                    dcl[o:o + n] = dl - b * 128
                    o += int(Tbr[b][r]) * 128
                gidx[:, col0:col0 + ncols] = _wrap16(idx)
                s_off = sg["slot0"] + chunk_off
                didx[:, s_off * 8:(s_off + T_r) * 8] = _wrap16(dloc)
                dcol[:, s_off:s_off + T_r] = \
                    np.ascontiguousarray(
                        dcl.reshape(T_r, 128).T).astype(np.int8)
        cores.append(dict(gidx=gidx, didx=didx, dcol=dcol))
    return sgs, cores, T_total, CG


def preprocess(cfg, edge_index):
    N, NC = cfg.N, cfg.NC
    src = np.concatenate([np.asarray(edge_index[0], np.int64),
                          np.arange(N, dtype=np.int64)])
    dst = np.concatenate([np.asarray(edge_index[1], np.int64),
                          np.arange(N, dtype=np.int64)])
    order = np.argsort(dst, kind="stable")
    src, dst = src[order], dst[order]
    cb = np.searchsorted(dst, [c * cfg.SHARD_PAD for c in range(NC + 1)])
    src_c = [src[cb[c]:cb[c + 1]] for c in range(NC)]
    dst_c = [dst[cb[c]:cb[c + 1]] for c in range(NC)]

    # both layers gather from chunk-major tables (contiguous AllGather
    # outputs); table position of node n = chunk-major permutation
    def pos2(n):
        c = n // cfg.SHARD_PAD
        l = n % cfg.SHARD_PAD
        return (l // cfg.CBR) * (NC * cfg.CBR) + c * cfg.CBR + (l % cfg.CBR)
    srcp = [pos2(s) for s in src_c]
    # both layers share the same tile structure and index tables
    sgs, cores, T, CG = _layer_meta(cfg, srcp, dst_c)

    class Meta:
        pass
    m = Meta()
    m.sgs, m.T, m.CG = sgs, T, CG
    per = cfg.NBLK // cfg.AG_CHUNKS
    m.ag = [(k, k + per) for k in range(0, cfg.NBLK, per)]
    return m, cores


# -------------------------------------------------------------- program -----
def build_program(cfg, meta, dbg=False, variant=()):
    maxg = 10**9
    for v in variant:
        if v.startswith("maxg="):
            maxg = int(v.split("=")[1])
    gcount = [0]
    import concourse.bass as bass
    import concourse.tile as tile
    from concourse import bacc, mybir

    bf16 = mybir.dt.bfloat16
    f32 = mybir.dt.float32
    i16 = mybir.dt.int16
    i8 = mybir.dt.int8
    AL = mybir.AluOpType
    AF = mybir.ActivationFunctionType

    N, Wd, C, NC = cfg.N, cfg.W, cfg.C, cfg.NC

    nc = bacc.Bacc("TRN2", target_bir_lowering=False, debug=False,
                   num_devices=NC)

    # packed constants: [w1aug(130) | w2aug(130) | wlin(C) | iota(128) | ident(128)]
    CPACK = 130 + 130 + C + 128 + 128
    cpack = nc.dram_tensor("cpack", [128, CPACK], f32, kind="ExternalInput")
    xTloc = nc.dram_tensor("xTloc", [128, cfg.SHARD_PAD], bf16,
                           kind="ExternalInput")
    # packed per-edge i16 tables (shared by both layers): [gidx(CG) | didx(T*8)]
    itab = nc.dram_tensor("itab", [16, meta.CG + meta.T * 8], i16,
                          kind="ExternalInput")
    dcol8 = nc.dram_tensor("dcol8", [128, meta.T], i8, kind="ExternalInput")
    out = nc.dram_tensor("out", [cfg.SHARD_PAD, C], bf16,
                         kind="ExternalOutput")

    kd = dict(kind="ExternalOutput") if dbg else {}
    hg1 = nc.dram_tensor("hg1", [cfg.NPAD, Wd], f32, addr_space="Shared")
    hg2 = nc.dram_tensor("hg2", [cfg.NPAD, Wd], f32, addr_space="Shared")
    ad1tab = nc.dram_tensor("ad1tab", [cfg.SHARD_PAD, cfg.ADW], f32)
    ad2tab = nc.dram_tensor("ad2tab", [cfg.SHARD_PAD, cfg.ADW], f32)
    shard1 = nc.dram_tensor("shard1", [cfg.SHARD_PAD, Wd], f32, **kd)
    shard2 = nc.dram_tensor("shard2", [cfg.SHARD_PAD, Wd], f32)
    if dbg:
        nt0 = meta.sgs[0]["nt"]
        dbghb = nc.dram_tensor("dbghb", [128, nt0, Wd], f32,
                               kind="ExternalOutput")
        dbgex = nc.dram_tensor("dbgex", [128, nt0], f32,
                               kind="ExternalOutput")

    groups = [list(range(NC))]

    with tile.TileContext(nc) as tc:
        cpool = tc.alloc_tile_pool(name="consts", bufs=1)
        cst = cpool.tile([128, CPACK], f32)
        nc.sync.dma_start(out=cst[:], in_=cpack[:])
        w1_s = cst[:, 0:130]
        w2_s = cst[:, 130:260]
        wl_s = cst[:, 260:260 + C]
        io_s = cst[:, 260 + C:388 + C]
        id_s = cst[:, 388 + C:516 + C]
        w1b = cpool.tile([128, 130], bf16)  # bf16 W1 for the bf16 x matmul
        nc.vector.tensor_copy(out=w1b[:], in_=w1_s)
        ad1c = cpool.tile([128, cfg.NBLK], f32)
        ad2c = cpool.tile([128, cfg.NBLK], f32)

        sb = tc.alloc_tile_pool(name="sb", bufs=3)
        gb = tc.alloc_tile_pool(name="gb", bufs=2 if "gb2" in variant else 1)
        eb = tc.alloc_tile_pool(name="eb", bufs=3)
        ps_a = tc.alloc_tile_pool(name="ps_a", bufs=2, space="PSUM")
        ps_g = tc.alloc_tile_pool(name="ps_g", bufs=3, space="PSUM")
        ps_o = tc.alloc_tile_pool(name="ps_o", bufs=1, space="PSUM")

        def col_tab_dma(tab, cols):
            """DMA [128, NBLK] col tile -> tab rows (row t*128+p, col 0)."""
            v = tab[:].rearrange("(t p) w -> p t w", p=128)
            nc.sync.dma_start(out=v[:, :, 0:1], in_=cols[:, :, None])

        def rep16(dst_tile, col0, ncols):
            """Rebuild the 8x partition replication of a [16, *] idx table."""
            nc.sync.dma_start(
                out=dst_tile[:, :ncols],
                in_=itab[None, :, col0:col0 + ncols].broadcast_to(
                    [8, 16, ncols]))

        # ones column (col 129) of both shard tables, written once
        onesc = cpool.tile([128, cfg.NBLK], f32)
        nc.vector.memset(onesc[:], 1.0)
        for _sh in (shard1, shard2):
            v = _sh[:].rearrange("(t p) w -> p t w", p=128)
            nc.sync.dma_start(out=v[:, :, 129:130], in_=onesc[:, :, None])

        # -------- phase A: local h1 shard + chunked AllGather -> hg1 ---------
        ag1_done = set()
        for b in (range(0) if "skipA" in variant else range(cfg.NBLK)):
            xt = sb.tile([128, 128], bf16, tag="xt")
            nc.sync.dma_start(out=xt[:], in_=xTloc[:, b * 128:(b + 1) * 128])
            ps = ps_a.tile([128, 130], f32, tag="psa")
            nc.tensor.matmul(ps[:], lhsT=xt[:], rhs=w1b[:],
                             start=True, stop=True, skip_group_check=True)
            hgt = sb.tile([128, Wd], f32, tag="hgt")
            nc.scalar.activation(out=hgt[:, 0:129], in_=ps[:, 0:129],
                                 func=AF.Copy)
            nc.vector.tensor_copy(out=ad1c[:, b:b + 1], in_=ps[:, 129:130])
            nc.sync.dma_start(out=shard1[b * 128:(b + 1) * 128, 0:129],
                              in_=hgt[:, 0:129])
            for k, (b0, b1) in enumerate(meta.ag):
                if b == b1 - 1 and k not in ag1_done:
                    ag1_done.add(k)
                    r0, r1 = b0 * 128, b1 * 128
                    g0 = k * NC * cfg.CBR
                    g1 = g0 + NC * cfg.CBR
                    if "noag" not in variant:
                        nc.gpsimd.collective_compute(
                            "AllGather", AL.bypass, replica_groups=groups,
                            ins=[shard1[r0:r1, :]],
                            outs=[hg1[g0:g1, :]])
                    else:
                        nc.sync.dma_start(out=hg1[g0:g0 + cfg.CBR, :],
                                          in_=shard1[r0:r0 + cfg.CBR, :])
        if "skipA" not in variant:
            col_tab_dma(ad1tab, ad1c)

        # ---------------- edge phase (shared for both layers) ----------------
        def edge_layer(sgs, hg_table, adtab, epilogue, layer):
            for sgi, sg in enumerate(sgs):
                t0, nt = sg["slot0"], sg["nt"]
                gcol0 = sg["calls"][0][0]
                gcols = sum(cl[1] for cl in sg["calls"])
                gix = gb.tile([128, max(gcols, 8)], i16, tag="gix")
                if gcols:
                    rep16(gix, gcol0, gcols)
                dix = gb.tile([128, nt * 8], i16, tag="dix")
                rep16(dix, meta.CG + t0 * 8, nt * 8)
                dcl8 = gb.tile([128, nt], i8, tag="dcl8")
                nc.sync.dma_start(out=dcl8[:], in_=dcol8[:, t0:t0 + nt])
                dcl = gb.tile([128, nt], f32, tag="dcl")
                nc.vector.tensor_copy(out=dcl[:], in_=dcl8[:])

                hbuf = gb.tile([128, nt, Wd], f32, tag="hbuf")
                for r in range(cfg.NR):
                    col0, ncols, T_r, chunk_off = sg["calls"][r]
                    if T_r == 0:
                        continue
                    gcount[0] += 1
                    if ("nogather" in variant or f"nogather{layer}" in variant
                            or gcount[0] > maxg):
                        nc.gpsimd.memset(hbuf[:, chunk_off:chunk_off + T_r, :], 0.5)
                    else:
                        nc.gpsimd.dma_gather(
                            out_ap=hbuf[:, chunk_off:chunk_off + T_r, :],
                            in_ap=hg_table[r * RANGE:
                                           min((r + 1) * RANGE, cfg.NPAD), :],
                            idxs_ap=gix[:, col0 - gcol0:col0 - gcol0 + ncols],
                            num_idxs=T_r * 128, num_idxs_reg=T_r * 128,
                            elem_size=Wd, single_packet=False)
                adb = gb.tile([128, nt, cfg.ADW], f32, tag="adb")
                gcount[0] += 1
                if ("nogather" in variant or "noadg" in variant
                        or f"nogather{layer}" in variant or gcount[0] > maxg):
                    nc.gpsimd.memset(adb[:], 0.25)
                else:
                    nc.gpsimd.dma_gather(
                        out_ap=adb[:], in_ap=adtab[:],
                        idxs_ap=dix[:], num_idxs=nt * 128, num_idxs_reg=nt * 128,
                        elem_size=cfg.ADW, single_packet=False)

                if "bare" in variant:
                    bc = sb.tile([128, Wd], f32, tag="barec")
                    nc.vector.tensor_copy(out=bc[:], in_=hbuf[:, 0, :])
                    nc.sync.dma_start(
                        out=shard2[sgi * 128:(sgi + 1) * 128, :], in_=bc[:])
                    continue
                ex = eb.tile([128, nt], f32, tag="ex")
                tmp = eb.tile([128, nt], f32, tag="tmp")
                if "noex" in variant:
                    nc.vector.memset(ex[:], 1.0)
                else:
                    nc.vector.tensor_tensor(out=ex[:], in0=hbuf[:, :, 0],
                                            in1=adb[:, :, 0], op=AL.add)
                    nc.vector.tensor_scalar(out=tmp[:], in0=ex[:],
                                            scalar1=cfg.neg, scalar2=None,
                                            op0=AL.mult)
                    nc.vector.tensor_tensor(out=ex[:], in0=ex[:], in1=tmp[:],
                                            op=AL.max)
                    nc.scalar.activation(out=ex[:], in_=ex[:], func=AF.Exp)
                # one-hot-times-ex for ALL tiles of the supergroup in two
                # DVE ops (broadcast APs), instead of one per tile
                sex = gb.tile([128, nt, 128], f32, tag="sex")
                if "nosex" not in variant:
                    nc.vector.tensor_tensor(
                        out=sex[:],
                        in0=io_s[:, None, :].broadcast_to([128, nt, 128]),
                        in1=dcl[:, :, None].broadcast_to([128, nt, 128]),
                        op=AL.is_equal)
                    nc.vector.tensor_tensor(
                        out=sex[:], in0=sex[:],
                        in1=ex[:, :, None].broadcast_to([128, nt, 128]),
                        op=AL.mult)
                if dbg and layer == 1 and sgi == 0:
                    nc.sync.dma_start(out=dbghb[:], in_=hbuf[:])
                    nc.sync.dma_start(out=dbgex[:], in_=ex[:])
                for (b, tslots) in sg["blocks"]:
                    ps = ps_g.tile([128, 129], f32, tag="psg")
                    ts = tslots[:1] if "nomm" in variant else tslots
                    for k, i in enumerate(ts):
                        sex_ap = io_s if "nosex" in variant else sex[:, i, :]
                        nc.tensor.matmul(ps[:], lhsT=sex_ap,
                                         rhs=hbuf[:, i, 1:130],
                                         start=(k == 0),
                                         stop=(k == len(ts) - 1),
                                         skip_group_check=True)
                    epilogue(b, ps)

        # ---------------- layer-1 epilogue: h2@W2 shard rows -----------------
        ag_done = set()

        def epi1(b, ps):
            # den > 0 always: every node has a self-loop edge
            rec = eb.tile([128, 1], f32, tag="rec")
            nc.vector.reciprocal(rec[:], ps[:, 128:129])
            h2b = sb.tile([128, 128], f32, tag="h2b")
            nc.scalar.activation(out=h2b[:], in_=ps[:, 0:128],
                                 func=AF.Relu, scale=rec[:])
            pst = ps_o.tile([128, 128], f32, tag="pst")
            nc.tensor.transpose(out=pst[:], in_=h2b[:], identity=id_s)
            h2t = sb.tile([128, 128], f32, tag="h3t")
            nc.scalar.activation(out=h2t[:], in_=pst[:], func=AF.Copy)
            ps2 = ps_o.tile([128, 130], f32, tag="pso")
            nc.tensor.matmul(ps2[:], lhsT=h2t[:], rhs=w2_s,
                             start=True, stop=True, skip_group_check=True)
            hg2t = sb.tile([128, Wd], f32, tag="hg2t")
            nc.scalar.activation(out=hg2t[:, 0:129], in_=ps2[:, 0:129],
                                 func=AF.Copy)
            nc.vector.tensor_copy(out=ad2c[:, b:b + 1], in_=ps2[:, 129:130])
            nc.sync.dma_start(out=shard2[b * 128:(b + 1) * 128, 0:129],
                              in_=hg2t[:, 0:129])
            for k, (b0, b1) in enumerate(meta.ag):
                if b == b1 - 1 and k not in ag_done:
                    ag_done.add(k)
                    r0, r1 = b0 * 128, b1 * 128
                    g0 = k * NC * cfg.CBR
                    g1 = g0 + NC * cfg.CBR
                    if "noag" not in variant:
                        nc.gpsimd.collective_compute(
                            "AllGather", AL.bypass, replica_groups=groups,
                            ins=[shard2[r0:r1, :]],
                            outs=[hg2[g0:g1, :]])
                    else:
                        nc.sync.dma_start(out=hg2[g0:g0 + cfg.CBR, :],
                                          in_=shard2[r0:r0 + cfg.CBR, :])

        # ---------------- layer-2 epilogue: classifier -----------------------
        def epi2(b, ps):
            rec = eb.tile([128, 1], f32, tag="rec")
            nc.vector.reciprocal(rec[:], ps[:, 128:129])
            h3 = sb.tile([128, 128], f32, tag="h2b")
            nc.scalar.activation(out=h3[:], in_=ps[:, 0:128],
                                 func=AF.Relu, scale=rec[:])
            pst = ps_o.tile([128, 128], f32, tag="pst")
            nc.tensor.transpose(out=pst[:], in_=h3[:], identity=id_s)
            h3t = sb.tile([128, 128], f32, tag="h3t")
            nc.scalar.activation(out=h3t[:], in_=pst[:], func=AF.Copy)
            pso = ps_o.tile([128, C], f32, tag="pso")
            nc.tensor.matmul(pso[:], lhsT=h3t[:], rhs=wl_s,
                             start=True, stop=True, skip_group_check=True)
            oc = sb.tile([128, C], bf16, tag="oc")
            nc.vector.tensor_copy(out=oc[:], in_=pso[:])
            nc.sync.dma_start(out=out[b * 128:(b + 1) * 128, :], in_=oc[:])

        edge_layer(meta.sgs, hg1, ad1tab, epi1, layer=1)
        if "bare" not in variant:
            col_tab_dma(ad2tab, ad2c)
            edge_layer(meta.sgs, hg2, ad2tab, epi2, layer=2)
        else:
            for b in range(cfg.NBLK):
                oc = sb.tile([128, C], bf16, tag="oc")
                nc.vector.memset(oc[:], 0.0)
                nc.sync.dma_start(out=out[b * 128:(b + 1) * 128, :], in_=oc[:])

        for _p in (ps_o, ps_g, ps_a, eb, gb, sb, cpool):
            _p.release()

    nc.compile()
    return nc


# ---------------------------------------------------------- input packing ---
def make_in_maps(cfg, meta, cores, inputs):
    import ml_dtypes
    bf = ml_dtypes.bfloat16
    x = np.asarray(inputs["x"], dtype=np.float32)
    W1 = np.asarray(inputs["W1"], dtype=np.float32)
    W2 = np.asarray(inputs["W2"], dtype=np.float32)
    Wl = np.asarray(inputs["Wlin"], dtype=np.float32)

    def aug(W, a_s, a_d):
        return np.concatenate(
            [(W @ a_s)[:, None], W, (W @ a_d)[:, None]], axis=1)

    C = cfg.C
    cpk = np.ascontiguousarray(np.concatenate([
        aug(W1, np.asarray(inputs["a_src1"], np.float32),
            np.asarray(inputs["a_dst1"], np.float32)),
        aug(W2, np.asarray(inputs["a_src2"], np.float32),
            np.asarray(inputs["a_dst2"], np.float32)),
        Wl,
        np.broadcast_to(np.arange(128, dtype=np.float32), (128, 128)),
        np.eye(128, dtype=np.float32),
    ], axis=1, dtype=np.float32))

    xTb = np.ascontiguousarray(x.T).astype(bf)
    maps = []
    for c in range(cfg.NC):
        lo = c * cfg.SHARD_PAD
        take = max(0, min(cfg.SHARD_PAD, cfg.N - lo))
        xl = np.zeros((128, cfg.SHARD_PAD), dtype=bf)
        xl[:, :take] = xTb[:, lo:lo + take]
        cc = cores[c]
        itab = np.concatenate([cc["gidx"], cc["didx"]], axis=1)
        maps.append(dict(cpack=cpk, xTloc=xl, itab=itab, dcol8=cc["dcol"]))
    return maps


# ------------------------------------------------------------------ entry ---
def kernel(**inputs) -> np.ndarray:
    from concourse.bass_utils import run_bass_kernel_spmd

    cfg = Cfg()
    meta, cores = preprocess(cfg, np.asarray(inputs["edge_index"]))
    nc = build_program(cfg, meta)
    in_maps = make_in_maps(cfg, meta, cores, inputs)
    res = run_bass_kernel_spmd(nc, in_maps, core_ids=list(range(cfg.NC)))
    outs = []
    for c in range(cfg.NC):
        take = min(cfg.SHARD_PAD, cfg.N - c * cfg.SHARD_PAD)
        outs.append(np.asarray(res.results[c]["out"])[:take])
    return np.concatenate(outs, axis=0).astype(np.float32)
